# revision 3
# baseline (speedup 1.0000x reference)
"""AttentiveItemToVec Trainium2 kernel (8 NeuronCores, batch-parallel).

Math (per batch row b):
  v = tvec_w[titems[b]]            [T,E]     (gather)
  u = cvec_w[citems[b]]            [C,E]     (gather)
  t_vec = v @ At_w.T + At_b        [T,DA]
  c_vec = u @ Ac_w.T + Ac_b        [C,DA]
  cos   = (t_vec/|t_vec|) . (c_vec/|c_vec|)   [T,C]
  attn  = softmax(mask(cos))       [T,C]
  z     = (attn @ (u @ Bc_w.T + Bc_b)) @ R_w.T + R_b
        = (attn@u) @ (R_w@Bc_w).T ... expanded here as:
          s = attn_unnorm @ u;  z = ((s/Sigma) @ Bc_w.T) @ R_w.T + (R_w@Bc_b + R_b)
  (softmax row-sums fold Bc_b through exactly since attn rows sum to 1)

Layout strategy per core (512 batch rows, groups of 16):
  - u gathered row-major [C,128] (c on partitions), PE-transposed to u_T [128,C]
  - c_vec computed DA-major [60,C]; cn^2 via ones-matmul (C-major out)
  - cos/softmax entirely C-major; exp does (num*invcn + masklog) in one ACT op
  - s_T accumulated E-major; group-level z matmuls; final transpose + 1/Sigma
"""

import os
import numpy as np
import ml_dtypes

import concourse.bass as bass
import concourse.bacc as bacc
import concourse.mybir as mybir
import concourse.tile as tile
from concourse.bass_utils import run_bass_kernel_spmd
from concourse.masks import make_identity

F32 = mybir.dt.float32
BF16 = mybir.dt.bfloat16
I32 = mybir.dt.int32
AF = mybir.ActivationFunctionType
OP = mybir.AluOpType

V, E, DA = 100000, 128, 60
B, T, C = 4096, 8, 200
NCORES = 8
BL = B // NCORES          # 512 local batch rows
NB = 16                   # batch rows per group (NB*T = 128 partitions)
NG = BL // NB             # 32 groups
C1, C2 = 128, C - 128     # C chunking: 128 + 72
NEG = -1e30

_CACHE: dict = {}


def _build():
    nc = bacc.Bacc(
        "TRN2", target_bir_lowering=False, debug=False, num_devices=NCORES
    )
    d = {}
    def din(name, shape, dt):
        d[name] = nc.dram_tensor(name, list(shape), dt, kind="ExternalInput").ap()
    din("tvec", [V, E], F32)
    din("cvec", [V, E], F32)
    din("acwt", [E, DA], BF16)      # Ac_w.T
    din("atwt", [E, DA], BF16)      # At_w.T
    din("bcwt", [E, E], BF16)       # Bc_w.T
    din("rwt", [E, E], BF16)        # R_w.T
    din("rwt32", [E, E], F32)       # R_w.T fp32 (c2 path)
    din("acb", [DA, 1], F32)
    din("atb", [DA, 1], F32)
    din("bcb32", [E, 1], F32)
    din("rb32", [E, 1], F32)
    din("cit1", [C1, BL], I32)
    din("cit2", [C2, BL], I32)
    din("mlog1", [C1, BL], F32)
    din("mlog2", [C2, BL], F32)
    din("titg", [NB * T, NG], I32)
    z_dram = nc.dram_tensor("z_out", [BL * T, E], F32, kind="ExternalOutput").ap()

    with tile.TileContext(nc) as tc:
        with (
            tc.tile_pool(name="const", bufs=1) as cp,
            tc.tile_pool(name="work", bufs=2) as wp,
            tc.tile_pool(name="work3", bufs=3) as wp3,
            tc.tile_pool(name="psA", bufs=2, space="PSUM") as psA,
            tc.tile_pool(name="psB", bufs=1, space="PSUM") as psB,
        ):
            # ---- constants into SBUF ----
            idb = cp.tile([128, 128], BF16, tag="idb")
            make_identity(nc, idb[:])
            idf = cp.tile([128, 128], F32, tag="idf")
            make_identity(nc, idf[:])
            onesb = cp.tile([128, 1], BF16, tag="onesb")
            nc.gpsimd.memset(onesb[:], 1.0)
            ones_row32 = cp.tile([1, 128], F32, tag="onesr")
            nc.gpsimd.memset(ones_row32[:], 1.0)

            acwt = cp.tile([E, DA], BF16, tag="acwt")
            nc.sync.dma_start(acwt[:], d["acwt"][:])
            atwt = cp.tile([E, DA], BF16, tag="atwt")
            nc.sync.dma_start(atwt[:], d["atwt"][:])
            bcwt = cp.tile([E, E], BF16, tag="bcwt")
            nc.sync.dma_start(bcwt[:], d["bcwt"][:])
            rwt = cp.tile([E, E], BF16, tag="rwt")
            nc.sync.dma_start(rwt[:], d["rwt"][:])
            rwt32 = cp.tile([E, E], F32, tag="rwt32")
            nc.sync.dma_start(rwt32[:], d["rwt32"][:])
            acb = cp.tile([DA, 1], F32, tag="acb")
            nc.sync.dma_start(acb[:], d["acb"][:])
            atb = cp.tile([DA, 1], F32, tag="atb")
            nc.sync.dma_start(atb[:], d["atb"][:])
            bcb32 = cp.tile([E, 1], F32, tag="bcb32")
            nc.sync.dma_start(bcb32[:], d["bcb32"][:])
            rb32 = cp.tile([E, 1], F32, tag="rb32")
            nc.sync.dma_start(rb32[:], d["rb32"][:])
            cit1 = cp.tile([C1, BL], I32, tag="cit1")
            nc.sync.dma_start(cit1[:], d["cit1"][:])
            cit2 = cp.tile([C2, BL], I32, tag="cit2")
            nc.sync.dma_start(cit2[:], d["cit2"][:])
            mlog1 = cp.tile([C1, BL], F32, tag="mlog1")
            nc.sync.dma_start(mlog1[:], d["mlog1"][:])
            mlog2 = cp.tile([C2, BL], F32, tag="mlog2")
            nc.sync.dma_start(mlog2[:], d["mlog2"][:])
            titg = cp.tile([NB * T, NG], I32, tag="titg")
            nc.sync.dma_start(titg[:], d["titg"][:])

            # ---- one-time: c2b = broadcast(R_w @ Bc_b + R_b) (fp32 path) ----
            ps_c2 = psB.tile([E, 1], F32, space="PSUM", tag="grp")
            nc.tensor.matmul(ps_c2[:], lhsT=rwt32[:], rhs=bcb32[:])
            c2col = cp.tile([E, 1], F32, tag="c2col")
            nc.scalar.activation(c2col[:], ps_c2[:], AF.Identity, bias=rb32[:])
            ps_c2r = psB.tile([1, E], F32, space="PSUM", tag="grp")
            nc.tensor.matmul(ps_c2r[:], lhsT=c2col[:], rhs=idf[:])
            c2row = cp.tile([1, E], F32, tag="c2row")
            nc.scalar.copy(c2row[:], ps_c2r[:])
            ps_c2b = psB.tile([E, E], F32, space="PSUM", tag="grp")
            nc.tensor.matmul(ps_c2b[:], lhsT=ones_row32[:], rhs=c2row[:])
            c2b = cp.tile([E, E], F32, tag="c2b")
            nc.scalar.copy(c2b[:], ps_c2b[:])

            # ---- main loop ----
            for g in range(NG):
                # --- t path (whole group: 16 b x 8 t = 128 rows) ---
                tv = wp.tile([128, E], BF16, tag="tv")
                nc.gpsimd.indirect_dma_start(
                    out=tv[:], out_offset=None, in_=d["tvec"][:],
                    in_offset=bass.IndirectOffsetOnAxis(ap=titg[:, g:g + 1], axis=0),
                )
                ps_vT = psB.tile([E, 128], F32, space="PSUM", tag="grp")
                nc.tensor.matmul(ps_vT[:], lhsT=tv[:], rhs=idb[:])
                vT = wp.tile([E, 128], BF16, tag="vT")
                nc.scalar.copy(vT[:], ps_vT[:])
                ps_tvT = psB.tile([DA, 128], F32, space="PSUM", tag="grp")
                nc.tensor.matmul(ps_tvT[:], lhsT=atwt[:], rhs=vT[:])
                tvT = wp.tile([DA, 128], BF16, tag="tvT")
                nc.scalar.activation(tvT[:], ps_tvT[:], AF.Identity, bias=atb[:])
                ps_tv = psB.tile([128, DA], F32, space="PSUM", tag="grp")
                nc.tensor.matmul(ps_tv[:], lhsT=tvT[:], rhs=idb[0:DA, 0:DA])
                tsq = wp.tile([128, DA], BF16, tag="tsq")
                tn2 = wp.tile([128, 1], F32, tag="tn2")
                nc.scalar.activation(
                    tsq[:], ps_tv[:], AF.Square, accum_out=tn2[:],
                )
                tnr = wp.tile([128, 1], F32, tag="tnr")
                nc.scalar.sqrt(tnr[:], tn2[:])
                invtn = wp.tile([128, 1], F32, tag="invtn")
                nc.vector.reciprocal(invtn[:], tnr[:])
                thbt = wp.tile([128, DA], BF16, tag="thbt")
                nc.vector.tensor_scalar_mul(thbt[:], ps_tv[:], invtn[:])
                ps_thT = psB.tile([DA, 128], F32, space="PSUM", tag="grp")
                nc.tensor.matmul(ps_thT[:], lhsT=thbt[:], rhs=idb[:])
                thT = wp.tile([DA, 128], BF16, tag="thT")
                nc.scalar.copy(thT[:], ps_thT[:])

                agA = wp.tile([C1, 128], BF16, tag="agA")
                agB = wp.tile([C2, 128], BF16, tag="agB")
                sTG = wp.tile([E, 128], BF16, tag="sTG")

                for i in range(NB):
                    b = g * NB + i
                    # gathers (fp32 table -> bf16 tiles, SWDGE cast)
                    u1 = wp3.tile([C1, E], BF16, tag="u1")
                    nc.gpsimd.indirect_dma_start(
                        out=u1[:], out_offset=None, in_=d["cvec"][:],
                        in_offset=bass.IndirectOffsetOnAxis(
                            ap=cit1[:, b:b + 1], axis=0),
                    )
                    u2 = wp3.tile([C2, E], BF16, tag="u2")
                    nc.gpsimd.indirect_dma_start(
                        out=u2[:], out_offset=None, in_=d["cvec"][:],
                        in_offset=bass.IndirectOffsetOnAxis(
                            ap=cit2[:, b:b + 1], axis=0),
                    )
                    # u_T = [u1; u2]^T  -> [E, C]
                    ps_uT = psA.tile([E, C], F32, space="PSUM", tag="uT")
                    nc.tensor.matmul(ps_uT[:, 0:C1], lhsT=u1[:], rhs=idb[:])
                    nc.tensor.matmul(ps_uT[:, C1:C], lhsT=u2[:], rhs=idb[0:C2, 0:C2])
                    uT = wp.tile([E, C], BF16, tag="uT_sb")
                    nc.scalar.copy(uT[:, 0:100], ps_uT[:, 0:100])
                    nc.vector.tensor_copy(uT[:, 100:C], ps_uT[:, 100:C])
                    # c_vec DA-major [60, C] (+bias on copy-out)
                    ps_cvT = psB.tile([DA, C], F32, space="PSUM", tag="cvT")
                    nc.tensor.matmul(ps_cvT[:], lhsT=acwt[:], rhs=uT[:])
                    cvT = wp.tile([DA, C], BF16, tag="cvT_sb")
                    nc.scalar.activation(cvT[:], ps_cvT[:], AF.Identity, bias=acb[:])
                    # cn^2 C-major via ones-matmul over squsquares
                    sq = wp.tile([DA, C], BF16, tag="sq")
                    nc.vector.scalar_tensor_tensor(
                        out=sq[:], in0=cvT[:], scalar=1.0, in1=cvT[:],
                        op0=OP.mult, op1=OP.mult,
                    )
                    ps_cn = psB.tile([C1, 2], F32, space="PSUM", tag="cn")
                    nc.tensor.matmul(ps_cn[:, 0:1], lhsT=sq[:, 0:C1],
                                     rhs=onesb[0:DA, :])
                    nc.tensor.matmul(ps_cn[0:C2, 1:2], lhsT=sq[:, C1:C],
                                     rhs=onesb[0:DA, :])
                    cnr1 = wp.tile([C1, 1], F32, tag="cnr1")
                    nc.scalar.sqrt(cnr1[:], ps_cn[:, 0:1])
                    cnr2 = wp.tile([C2, 1], F32, tag="cnr2")
                    nc.scalar.sqrt(cnr2[:], ps_cn[0:C2, 1:2])
                    icn1 = wp.tile([C1, 1], F32, tag="icn1")
                    nc.vector.reciprocal(icn1[:], cnr1[:])
                    icn2 = wp.tile([C2, 1], F32, tag="icn2")
                    nc.vector.reciprocal(icn2[:], cnr2[:])
                    # num C-major [C, 8]
                    ps_nT = psA.tile([C1, 2 * T], F32, space="PSUM", tag="nT")
                    nc.tensor.matmul(ps_nT[:, 0:T], lhsT=cvT[:, 0:C1],
                                     rhs=thT[:, i * T:(i + 1) * T])
                    nc.tensor.matmul(ps_nT[0:C2, T:2 * T], lhsT=cvT[:, C1:C],
                                     rhs=thT[:, i * T:(i + 1) * T])
                    # attn_unnorm = exp(num*invcn + masklog)
                    nc.scalar.activation(
                        agA[:, i * T:(i + 1) * T], ps_nT[:, 0:T], AF.Exp,
                        bias=mlog1[:, b:b + 1], scale=icn1[:],
                    )
                    nc.scalar.activation(
                        agB[:, i * T:(i + 1) * T], ps_nT[0:C2, T:2 * T], AF.Exp,
                        bias=mlog2[:, b:b + 1], scale=icn2[:],
                    )
                    # s_T = u^T @ attn  [E, 8]
                    ps_sT = psB.tile([E, T], F32, space="PSUM", tag="sT")
                    nc.tensor.matmul(ps_sT[:], lhsT=u1[:],
                                     rhs=agA[:, i * T:(i + 1) * T],
                                     start=True, stop=False)
                    nc.tensor.matmul(ps_sT[:], lhsT=u2[:],
                                     rhs=agB[:, i * T:(i + 1) * T],
                                     start=False, stop=True)
                    nc.vector.tensor_copy(sTG[:, i * T:(i + 1) * T], ps_sT[:])

                # --- group tail: Sigma, z path ---
                ps_sum = psB.tile([128, 1], F32, space="PSUM", tag="grp")
                nc.tensor.matmul(ps_sum[:], lhsT=agA[:], rhs=onesb[0:C1, :],
                                 start=True, stop=False)
                nc.tensor.matmul(ps_sum[:], lhsT=agB[:], rhs=onesb[0:C2, :],
                                 start=False, stop=True)
                invS = wp.tile([128, 1], F32, tag="invS")
                nc.vector.reciprocal(invS[:], ps_sum[:])

                ps_yT = psB.tile([E, 128], F32, space="PSUM", tag="grp")
                nc.tensor.matmul(ps_yT[:], lhsT=bcwt[:], rhs=sTG[:])
                yT = wp.tile([E, 128], BF16, tag="yT")
                nc.scalar.copy(yT[:], ps_yT[:])
                ps_zT = psB.tile([E, 128], F32, space="PSUM", tag="grp")
                nc.tensor.matmul(ps_zT[:], lhsT=rwt[:], rhs=yT[:])
                zT = wp.tile([E, 128], BF16, tag="zT")
                nc.scalar.copy(zT[:], ps_zT[:])
                ps_z = psB.tile([128, E], F32, space="PSUM", tag="grp")
                nc.tensor.matmul(ps_z[:], lhsT=zT[:], rhs=idb[:])
                zout = wp.tile([128, E], F32, tag="zout")
                nc.vector.scalar_tensor_tensor(
                    out=zout[:], in0=ps_z[:], scalar=invS[:], in1=c2b[:],
                    op0=OP.mult, op1=OP.add,
                )
                nc.sync.dma_start(z_dram[g * 128:(g + 1) * 128, :], zout[:])

    nc.compile()
    return nc


def _prep_core_inputs(inputs, k):
    bf = ml_dtypes.bfloat16
    sl = slice(k * BL, (k + 1) * BL)
    tit = np.ascontiguousarray(
        inputs["batch_titems"][sl].astype(np.int32).reshape(NG, NB * T).T)
    cit = inputs["batch_citems"][sl].astype(np.int32).T
    mlog = np.where(inputs["mask_pad_ids"][sl], NEG, 0.0).astype(np.float32).T
    m = {
        "tvec": np.asarray(inputs["tvec_w"], dtype=np.float32),
        "cvec": np.asarray(inputs["cvec_w"], dtype=np.float32),
        "acwt": np.ascontiguousarray(inputs["Ac_w"].T).astype(bf),
        "atwt": np.ascontiguousarray(inputs["At_w"].T).astype(bf),
        "bcwt": np.ascontiguousarray(inputs["Bc_w"].T).astype(bf),
        "rwt": np.ascontiguousarray(inputs["R_w"].T).astype(bf),
        "rwt32": np.ascontiguousarray(inputs["R_w"].T).astype(np.float32),
        "acb": np.asarray(inputs["Ac_b"], dtype=np.float32).reshape(DA, 1),
        "atb": np.asarray(inputs["At_b"], dtype=np.float32).reshape(DA, 1),
        "bcb32": np.asarray(inputs["Bc_b"], dtype=np.float32).reshape(E, 1),
        "rb32": np.asarray(inputs["R_b"], dtype=np.float32).reshape(E, 1),
        "cit1": np.ascontiguousarray(cit[0:C1]),
        "cit2": np.ascontiguousarray(cit[C1:C]),
        "mlog1": np.ascontiguousarray(mlog[0:C1]),
        "mlog2": np.ascontiguousarray(mlog[C1:C]),
        "titg": tit,
    }
    return m


def _install_profile_hook():
    """Dev-only: register the axon NTFF hook missing from this image."""
    import sys
    import types
    try:
        import antenv.axon_hooks  # noqa: F401
        return
    except ImportError:
        pass
    from trn_agent_boot.trn_boot import _ntff_profile_via_ctypes
    hook = _ntff_profile_via_ctypes("/opt/axon/libaxon_pjrt.so")
    mod = types.ModuleType("antenv.axon_hooks")
    mod._hook = hook
    mod.set_axon_ntff_profile_hook = lambda h: setattr(mod, "_hook", h)
    mod.get_axon_ntff_profile_hook = lambda: mod._hook
    sys.modules["antenv.axon_hooks"] = mod
    import antenv
    antenv.axon_hooks = mod


def kernel(**inputs) -> np.ndarray:
    if "nc" not in _CACHE:
        _CACHE["nc"] = _build()
    nc = _CACHE["nc"]
    inputs = {k: np.asarray(v) for k, v in inputs.items()}
    in_maps = [_prep_core_inputs(inputs, k) for k in range(NCORES)]
    trace = bool(int(os.environ.get("KERNEL_TRACE", "0")))
    kw = {}
    if trace:
        try:
            _install_profile_hook()
            import concourse.bass_utils as _bu
            _bu.upload_artifacts = lambda d: d
            tdir = os.environ.get("KERNEL_TRACE_DIR", "/root/problem/_trace")
            os.makedirs(tdir, exist_ok=True)
            kw["tmpdir"] = tdir
        except Exception as e:  # profiling is best-effort
            print(f"trace setup failed: {e}")
            trace = False
    res = run_bass_kernel_spmd(
        nc, in_maps, list(range(NCORES)), trace=trace, **kw,
    )
    _CACHE["last_result"] = res
    z = np.concatenate(
        [res.results[k]["z_out"].reshape(BL, T, E) for k in range(NCORES)], axis=0
    )
    return z.astype(np.float32)


# revision 11
# speedup vs baseline: 2.0045x; 2.0045x over previous
"""AttentiveItemToVec Trainium2 kernel (8 NeuronCores, batch-parallel).

Math (per batch row b):
  v = tvec_w[titems[b]]            [T,E]     (gather)
  u = cvec_w[citems[b]]            [C,E]     (gather)
  t_vec = v @ At_w.T + At_b        [T,DA]
  c_vec = u @ Ac_w.T + Ac_b        [C,DA]
  cos   = (t_vec/|t_vec|) . (c_vec/|c_vec|)   [T,C]
  attn  = softmax(mask(cos))       [T,C]
  z     = (attn @ (u @ Bc_w.T + Bc_b)) @ R_w.T + R_b
        = (attn@u) @ (R_w@Bc_w).T ... expanded here as:
          s = attn_unnorm @ u;  z = ((s/Sigma) @ Bc_w.T) @ R_w.T + (R_w@Bc_b + R_b)
  (softmax row-sums fold Bc_b through exactly since attn rows sum to 1)

Layout strategy per core (512 batch rows, groups of 16):
  - u gathered row-major [C,128] (c on partitions), PE-transposed to u_T [128,C]
  - c_vec computed DA-major [60,C]; cn^2 via ones-matmul (C-major out)
  - cos/softmax entirely C-major; exp does (num*invcn + masklog) in one ACT op
  - s_T accumulated E-major; group-level z matmuls; final transpose + 1/Sigma
"""

import os
import numpy as np
import ml_dtypes

import concourse.bass as bass
import concourse.bacc as bacc
import concourse.mybir as mybir
import concourse.tile as tile
from concourse.bass_utils import run_bass_kernel_spmd
from concourse.masks import make_identity

F32 = mybir.dt.float32
BF16 = mybir.dt.bfloat16
I32 = mybir.dt.int32
AF = mybir.ActivationFunctionType
OP = mybir.AluOpType

V, E, DA = 100000, 128, 60
B, T, C = 4096, 8, 200
NCORES = 8
BL = B // NCORES          # 512 local batch rows
NB = 16                   # batch rows per group (NB*T = 128 partitions)
NG = BL // NB             # 32 groups
PB = 4                    # batch rows gathered per indirect DMA
C1, C2 = 128, C - 128     # C chunking: 128 + 72
NEG = -1e30

_CACHE: dict = {}


def _pin_act_table():
    """Force every activation onto the natural_log_exp_and_others table.

    All ACT funcs used here (Copy/Identity/Square/Ln/Exp) live in that one
    table, but the table chooser picks the first table containing each
    function, which makes Exp->Ln sequences thrash 1.28us ACT_TABLE_LOADs.
    Emptying the other sets (names/positions preserved so act_func_set ids
    stay valid) pins the choice; one load total.
    """
    from concourse.hw_specs import get_activation_tables
    keep = "natural_log_exp_and_others"
    orig = get_activation_tables("gen3")
    pinned = {k: (v if k == keep else set()) for k, v in orig.items()}
    bacc.get_activation_tables = lambda arch: pinned


def _build():
    _pin_act_table()
    nc = bacc.Bacc(
        "TRN2", target_bir_lowering=False, debug=False, num_devices=NCORES
    )
    d = {}
    def din(name, shape, dt):
        d[name] = nc.dram_tensor(name, list(shape), dt, kind="ExternalInput").ap()
    din("tvec", [V, E], F32)
    din("cvec", [V, E], F32)
    din("acwt", [E, DA], BF16)      # Ac_w.T
    din("atwt", [E, DA], BF16)      # At_w.T
    din("bcwt", [E, E], BF16)       # Bc_w.T
    din("rwt", [E, E], BF16)        # R_w.T
    din("rwt32", [E, E], F32)       # R_w.T fp32 (c2 path)
    din("acb", [DA, 1], F32)
    din("atb", [DA, 1], F32)
    din("bcb32", [E, 1], F32)
    din("rb32", [E, 1], F32)
    din("cit1", [C1, BL], I32)
    din("cit2", [C2, BL], I32)
    din("mlog1", [C1, BL], F32)
    din("mlog2", [C2, BL], F32)
    din("titg", [NB * T, NG], I32)
    z_dram = nc.dram_tensor("z_out", [BL * T, E], F32, kind="ExternalOutput").ap()

    with tile.TileContext(nc) as tc:
        with (
            tc.tile_pool(name="const", bufs=1) as cp,
            tc.tile_pool(name="work", bufs=2) as wp,
            tc.tile_pool(name="work3", bufs=2 * PB) as wp3,
            tc.tile_pool(name="psA", bufs=2, space="PSUM") as psA,
            tc.tile_pool(name="psB", bufs=1, space="PSUM") as psB,
        ):
            # ---- constants into SBUF ----
            idb = cp.tile([128, 128], BF16, tag="idb")
            make_identity(nc, idb[:])
            idf = cp.tile([128, 128], F32, tag="idf")
            make_identity(nc, idf[:])
            onesb = cp.tile([128, 1], BF16, tag="onesb")
            nc.gpsimd.memset(onesb[:], 1.0)
            ones_row32 = cp.tile([1, 128], F32, tag="onesr")
            nc.gpsimd.memset(ones_row32[:], 1.0)

            acwt = cp.tile([E, DA], BF16, tag="acwt")
            nc.sync.dma_start(acwt[:], d["acwt"][:])
            atwt = cp.tile([E, DA], BF16, tag="atwt")
            nc.sync.dma_start(atwt[:], d["atwt"][:])
            bcwt = cp.tile([E, E], BF16, tag="bcwt")
            nc.sync.dma_start(bcwt[:], d["bcwt"][:])
            rwt = cp.tile([E, E], BF16, tag="rwt")
            nc.sync.dma_start(rwt[:], d["rwt"][:])
            rwt32 = cp.tile([E, E], F32, tag="rwt32")
            nc.sync.dma_start(rwt32[:], d["rwt32"][:])
            acb = cp.tile([DA, 1], F32, tag="acb")
            nc.sync.dma_start(acb[:], d["acb"][:])
            atb = cp.tile([DA, 1], F32, tag="atb")
            nc.sync.dma_start(atb[:], d["atb"][:])
            bcb32 = cp.tile([E, 1], F32, tag="bcb32")
            nc.sync.dma_start(bcb32[:], d["bcb32"][:])
            rb32 = cp.tile([E, 1], F32, tag="rb32")
            nc.sync.dma_start(rb32[:], d["rb32"][:])
            cit1 = cp.tile([C1, BL], I32, tag="cit1")
            nc.sync.dma_start(cit1[:], d["cit1"][:])
            cit2 = cp.tile([C2, BL], I32, tag="cit2")
            nc.sync.dma_start(cit2[:], d["cit2"][:])
            mlog1 = cp.tile([C1, BL], F32, tag="mlog1")
            nc.sync.dma_start(mlog1[:], d["mlog1"][:])
            mlog2 = cp.tile([C2, BL], F32, tag="mlog2")
            nc.sync.dma_start(mlog2[:], d["mlog2"][:])
            titg = cp.tile([NB * T, NG], I32, tag="titg")
            nc.sync.dma_start(titg[:], d["titg"][:])

            # ---- one-time: c2b = broadcast(R_w @ Bc_b + R_b) (fp32 path) ----
            ps_c2 = psB.tile([E, 1], F32, space="PSUM", tag="grp")
            nc.tensor.matmul(ps_c2[:], lhsT=rwt32[:], rhs=bcb32[:])
            c2col = cp.tile([E, 1], F32, tag="c2col")
            nc.scalar.activation(c2col[:], ps_c2[:], AF.Identity, bias=rb32[:])
            ps_c2r = psB.tile([1, E], F32, space="PSUM", tag="grp")
            nc.tensor.matmul(ps_c2r[:], lhsT=c2col[:], rhs=idf[:])
            c2row = cp.tile([1, E], F32, tag="c2row")
            nc.scalar.copy(c2row[:], ps_c2r[:])
            ps_c2b = psB.tile([E, E], F32, space="PSUM", tag="grp")
            nc.tensor.matmul(ps_c2b[:], lhsT=ones_row32[:], rhs=c2row[:])
            c2b = cp.tile([E, E], F32, tag="c2b")
            nc.scalar.copy(c2b[:], ps_c2b[:])

            # ---- main loop ----
            for g in range(NG):
                # --- t path (whole group: 16 b x 8 t = 128 rows) ---
                tv = wp.tile([128, E], BF16, tag="tv")
                nc.gpsimd.indirect_dma_start(
                    out=tv[:], out_offset=None, in_=d["tvec"][:],
                    in_offset=bass.IndirectOffsetOnAxis(ap=titg[:, g:g + 1], axis=0),
                )
                ps_vT = psB.tile([E, 128], F32, space="PSUM", tag="grp")
                nc.tensor.matmul(ps_vT[:], lhsT=tv[:], rhs=idb[:])
                vT = wp.tile([E, 128], BF16, tag="vT")
                nc.scalar.copy(vT[:], ps_vT[:])
                ps_tvT = psB.tile([DA, 128], F32, space="PSUM", tag="grp")
                nc.tensor.matmul(ps_tvT[:], lhsT=atwt[:], rhs=vT[:])
                tvT = wp.tile([DA, 128], BF16, tag="tvT")
                nc.scalar.activation(tvT[:], ps_tvT[:], AF.Identity, bias=atb[:])
                ps_tv = psB.tile([128, DA], F32, space="PSUM", tag="grp")
                nc.tensor.matmul(ps_tv[:], lhsT=tvT[:], rhs=idb[0:DA, 0:DA])
                tsq = wp.tile([128, DA], BF16, tag="tsq")
                tn2 = wp.tile([128, 1], F32, tag="tn2")
                nc.scalar.activation(
                    tsq[:], ps_tv[:], AF.Square, accum_out=tn2[:],
                )
                # 1/sqrt(x) = exp(-0.5*ln(x)): keeps every ACT func in the
                # natural_log_exp table (a Sqrt would force 1.3us table
                # reloads next to each Exp)
                ltn = wp.tile([128, 1], F32, tag="ltn")
                nc.scalar.activation(ltn[:], tn2[:], AF.Ln)
                invtn = wp.tile([128, 1], F32, tag="invtn")
                nc.scalar.activation(invtn[:], ltn[:], AF.Exp, scale=-0.5)
                thbt = wp.tile([128, DA], BF16, tag="thbt")
                nc.vector.tensor_scalar_mul(thbt[:], ps_tv[:], invtn[:])
                ps_thT = psB.tile([DA, 128], F32, space="PSUM", tag="grp")
                nc.tensor.matmul(ps_thT[:], lhsT=thbt[:], rhs=idb[:])
                thT = wp.tile([DA, 128], BF16, tag="thT")
                nc.scalar.copy(thT[:], ps_thT[:])

                agA = wp.tile([C1, 128], BF16, tag="agA")
                agB = wp.tile([C2, 128], BF16, tag="agB")
                sTG = wp.tile([E, 128], BF16, tag="sTG")

                for blk in range(NB // PB):
                    us = []
                    ps_nT4 = psA.tile([C1, PB * 18], F32, space="PSUM", tag="nT")
                    for j in range(PB):
                        i = blk * PB + j
                        b = g * NB + i
                        u1 = wp3.tile([C1, E], BF16, tag="u1")
                        nc.gpsimd.indirect_dma_start(
                            out=u1[:], out_offset=None, in_=d["cvec"][:],
                            in_offset=bass.IndirectOffsetOnAxis(
                                ap=cit1[:, b:b + 1], axis=0),
                        )
                        u2 = wp3.tile([C2, E], BF16, tag="u2")
                        nc.gpsimd.indirect_dma_start(
                            out=u2[:], out_offset=None, in_=d["cvec"][:],
                            in_offset=bass.IndirectOffsetOnAxis(
                                ap=cit2[:, b:b + 1], axis=0),
                        )
                        us.append((u1, u2))
                        # u_T = [u1; u2]^T  -> [E, C]
                        ps_uT = psA.tile([E, C], F32, space="PSUM", tag="uT")
                        nc.tensor.matmul(ps_uT[:, 0:C1], lhsT=u1[:], rhs=idb[:])
                        nc.tensor.matmul(ps_uT[:, C1:C], lhsT=u2[:],
                                         rhs=idb[0:C2, 0:C2])
                        uT = wp.tile([E, C], BF16, tag="uT_sb")
                        nc.scalar.copy(uT[:, 0:100], ps_uT[:, 0:100])
                        nc.vector.tensor_copy(uT[:, 100:C], ps_uT[:, 100:C])
                        # c_vec DA-major [60, C] (+bias via DVE on copy-out)
                        ps_cvT = psB.tile([DA, C], F32, space="PSUM", tag="cvT")
                        nc.tensor.matmul(ps_cvT[:], lhsT=acwt[:], rhs=uT[:])
                        cvT = wp.tile([DA, C], BF16, tag="cvT_sb")
                        nc.vector.tensor_scalar(
                            out=cvT[:], in0=ps_cvT[:], scalar1=acb[:],
                            scalar2=None, op0=OP.add,
                        )
                        sq = wp.tile([DA, C], BF16, tag="sq")
                        nc.vector.scalar_tensor_tensor(
                            out=sq[:], in0=cvT[:], scalar=1.0, in1=cvT[:],
                            op0=OP.mult, op1=OP.mult,
                        )
                        # per-b columns of ps_nT4: [18j,18j+8) num1,
                        # [18j+8,18j+16) num2 (rows<72), 18j+16 cn1, 18j+17 cn2
                        o = 18 * j
                        nc.tensor.matmul(ps_nT4[:, o + 16:o + 17],
                                         lhsT=sq[:, 0:C1], rhs=onesb[0:DA, :])
                        nc.tensor.matmul(ps_nT4[0:C2, o + 17:o + 18],
                                         lhsT=sq[:, C1:C], rhs=onesb[0:DA, :])
                        nc.tensor.matmul(ps_nT4[:, o:o + T], lhsT=cvT[:, 0:C1],
                                         rhs=thT[:, i * T:(i + 1) * T])
                        nc.tensor.matmul(ps_nT4[0:C2, o + T:o + 2 * T],
                                         lhsT=cvT[:, C1:C],
                                         rhs=thT[:, i * T:(i + 1) * T])
                    # batched invcn = exp(-0.5*ln(cn^2)) for all PB rows
                    lcn = wp.tile([C1, PB, 2], F32, tag="lcn")
                    cn_view = ps_nT4[:].rearrange("p (b k) -> p b k", k=18)[:, :, 16:18]
                    nc.scalar.activation(lcn[:], cn_view, AF.Ln)
                    invcn = wp.tile([C1, PB * 2], F32, tag="invcn")
                    nc.scalar.activation(
                        invcn[:], lcn[:].rearrange("p b k -> p (b k)"),
                        AF.Exp, scale=-0.5)
                    for j in range(PB):
                        i = blk * PB + j
                        b = g * NB + i
                        u1, u2 = us[j]
                        o = 18 * j
                        # attn_unnorm = exp(num*invcn + masklog)
                        nc.scalar.activation(
                            agA[:, i * T:(i + 1) * T], ps_nT4[:, o:o + T],
                            AF.Exp,
                            bias=mlog1[:, b:b + 1],
                            scale=invcn[:, 2 * j:2 * j + 1],
                        )
                        nc.scalar.activation(
                            agB[:, i * T:(i + 1) * T],
                            ps_nT4[0:C2, o + T:o + 2 * T], AF.Exp,
                            bias=mlog2[:, b:b + 1],
                            scale=invcn[0:C2, 2 * j + 1:2 * j + 2],
                        )
                        # s_T = u^T @ attn  [E, 8]
                        ps_sT = psB.tile([E, T], F32, space="PSUM", tag="sT")
                        nc.tensor.matmul(ps_sT[:], lhsT=u1[:],
                                         rhs=agA[:, i * T:(i + 1) * T],
                                         start=True, stop=False)
                        nc.tensor.matmul(ps_sT[:], lhsT=u2[:],
                                         rhs=agB[:, i * T:(i + 1) * T],
                                         start=False, stop=True)
                        nc.vector.tensor_copy(sTG[:, i * T:(i + 1) * T],
                                              ps_sT[:])

                # --- group tail: Sigma, z path ---
                ps_sum = psB.tile([128, 1], F32, space="PSUM", tag="grp")
                nc.tensor.matmul(ps_sum[:], lhsT=agA[:], rhs=onesb[0:C1, :],
                                 start=True, stop=False)
                nc.tensor.matmul(ps_sum[:], lhsT=agB[:], rhs=onesb[0:C2, :],
                                 start=False, stop=True)
                invS = wp.tile([128, 1], F32, tag="invS")
                nc.vector.reciprocal(invS[:], ps_sum[:])

                ps_yT = psB.tile([E, 128], F32, space="PSUM", tag="grp")
                nc.tensor.matmul(ps_yT[:], lhsT=bcwt[:], rhs=sTG[:])
                yT = wp.tile([E, 128], BF16, tag="yT")
                nc.scalar.copy(yT[:], ps_yT[:])
                ps_zT = psB.tile([E, 128], F32, space="PSUM", tag="grp")
                nc.tensor.matmul(ps_zT[:], lhsT=rwt[:], rhs=yT[:])
                zT = wp.tile([E, 128], BF16, tag="zT")
                nc.scalar.copy(zT[:], ps_zT[:])
                ps_z = psB.tile([128, E], F32, space="PSUM", tag="grp")
                nc.tensor.matmul(ps_z[:], lhsT=zT[:], rhs=idb[:])
                zout = wp.tile([128, E], F32, tag="zout")
                nc.vector.scalar_tensor_tensor(
                    out=zout[:], in0=ps_z[:], scalar=invS[:], in1=c2b[:],
                    op0=OP.mult, op1=OP.add,
                )
                nc.sync.dma_start(z_dram[g * 128:(g + 1) * 128, :], zout[:])

    nc.compile()
    return nc


def _prep_core_inputs(inputs, k):
    bf = ml_dtypes.bfloat16
    sl = slice(k * BL, (k + 1) * BL)
    tit = np.ascontiguousarray(
        inputs["batch_titems"][sl].astype(np.int32).reshape(NG, NB * T).T)
    cit = inputs["batch_citems"][sl].astype(np.int32).T
    mlog = np.where(inputs["mask_pad_ids"][sl], NEG, 0.0).astype(np.float32).T
    m = {
        "tvec": np.asarray(inputs["tvec_w"], dtype=np.float32),
        "cvec": np.asarray(inputs["cvec_w"], dtype=np.float32),
        "acwt": np.ascontiguousarray(inputs["Ac_w"].T).astype(bf),
        "atwt": np.ascontiguousarray(inputs["At_w"].T).astype(bf),
        "bcwt": np.ascontiguousarray(inputs["Bc_w"].T).astype(bf),
        "rwt": np.ascontiguousarray(inputs["R_w"].T).astype(bf),
        "rwt32": np.ascontiguousarray(inputs["R_w"].T).astype(np.float32),
        "acb": np.asarray(inputs["Ac_b"], dtype=np.float32).reshape(DA, 1),
        "atb": np.asarray(inputs["At_b"], dtype=np.float32).reshape(DA, 1),
        "bcb32": np.asarray(inputs["Bc_b"], dtype=np.float32).reshape(E, 1),
        "rb32": np.asarray(inputs["R_b"], dtype=np.float32).reshape(E, 1),
        "cit1": np.ascontiguousarray(cit[0:C1]),
        "cit2": np.ascontiguousarray(cit[C1:C]),
        "mlog1": np.ascontiguousarray(mlog[0:C1]),
        "mlog2": np.ascontiguousarray(mlog[C1:C]),
        "titg": tit,
    }
    return m


def _install_profile_hook():
    """Dev-only: register the axon NTFF hook missing from this image."""
    import sys
    import types
    try:
        import antenv.axon_hooks  # noqa: F401
        return
    except ImportError:
        pass
    from trn_agent_boot.trn_boot import _ntff_profile_via_ctypes
    hook = _ntff_profile_via_ctypes("/opt/axon/libaxon_pjrt.so")
    mod = types.ModuleType("antenv.axon_hooks")
    mod._hook = hook
    mod.set_axon_ntff_profile_hook = lambda h: setattr(mod, "_hook", h)
    mod.get_axon_ntff_profile_hook = lambda: mod._hook
    sys.modules["antenv.axon_hooks"] = mod
    import antenv
    antenv.axon_hooks = mod


def kernel(**inputs) -> np.ndarray:
    if "nc" not in _CACHE:
        _CACHE["nc"] = _build()
    nc = _CACHE["nc"]
    inputs = {k: np.asarray(v) for k, v in inputs.items()}
    in_maps = [_prep_core_inputs(inputs, k) for k in range(NCORES)]
    trace = bool(int(os.environ.get("KERNEL_TRACE", "0")))
    kw = {}
    if trace:
        try:
            _install_profile_hook()
            import concourse.bass_utils as _bu
            _bu.upload_artifacts = lambda d: d
            tdir = os.environ.get("KERNEL_TRACE_DIR", "/root/problem/_trace")
            os.makedirs(tdir, exist_ok=True)
            kw["tmpdir"] = tdir
        except Exception as e:  # profiling is best-effort
            print(f"trace setup failed: {e}")
            trace = False
    res = run_bass_kernel_spmd(
        nc, in_maps, list(range(NCORES)), trace=trace, **kw,
    )
    _CACHE["last_result"] = res
    z = np.concatenate(
        [res.results[k]["z_out"].reshape(BL, T, E) for k in range(NCORES)], axis=0
    )
    return z.astype(np.float32)


# revision 12
# speedup vs baseline: 2.1476x; 1.0714x over previous
"""AttentiveItemToVec Trainium2 kernel (8 NeuronCores, batch-parallel).

Math (per batch row b):
  v = tvec_w[titems[b]]            [T,E]     (gather)
  u = cvec_w[citems[b]]            [C,E]     (gather)
  t_vec = v @ At_w.T + At_b        [T,DA]
  c_vec = u @ Ac_w.T + Ac_b        [C,DA]
  cos   = (t_vec/|t_vec|) . (c_vec/|c_vec|)   [T,C]
  attn  = softmax(mask(cos))       [T,C]
  z     = (attn @ (u @ Bc_w.T + Bc_b)) @ R_w.T + R_b
        = (attn@u) @ (R_w@Bc_w).T ... expanded here as:
          s = attn_unnorm @ u;  z = ((s/Sigma) @ Bc_w.T) @ R_w.T + (R_w@Bc_b + R_b)
  (softmax row-sums fold Bc_b through exactly since attn rows sum to 1)

Layout strategy per core (512 batch rows, groups of 16):
  - u gathered row-major [C,128] (c on partitions), PE-transposed to u_T [128,C]
  - c_vec computed DA-major [60,C]; cn^2 via ones-matmul (C-major out)
  - cos/softmax entirely C-major; exp does (num*invcn + masklog) in one ACT op
  - s_T accumulated E-major; group-level z matmuls; final transpose + 1/Sigma
"""

import os
import numpy as np
import ml_dtypes

import concourse.bass as bass
import concourse.bacc as bacc
import concourse.mybir as mybir
import concourse.tile as tile
from concourse.bass_utils import run_bass_kernel_spmd
from concourse.masks import make_identity

F32 = mybir.dt.float32
BF16 = mybir.dt.bfloat16
I32 = mybir.dt.int32
AF = mybir.ActivationFunctionType
OP = mybir.AluOpType

V, E, DA = 100000, 128, 60
B, T, C = 4096, 8, 200
NCORES = 8
BL = B // NCORES          # 512 local batch rows
NB = 16                   # batch rows per group (NB*T = 128 partitions)
NG = BL // NB             # 32 groups
PB = 4                    # batch rows gathered per indirect DMA
C1, C2 = 128, C - 128     # C chunking: 128 + 72
NEG = -1e30

_CACHE: dict = {}


def _pin_act_table():
    """Force every activation onto the natural_log_exp_and_others table.

    All ACT funcs used here (Copy/Identity/Square/Ln/Exp) live in that one
    table, but the table chooser picks the first table containing each
    function, which makes Exp->Ln sequences thrash 1.28us ACT_TABLE_LOADs.
    Emptying the other sets (names/positions preserved so act_func_set ids
    stay valid) pins the choice; one load total.
    """
    from concourse.hw_specs import get_activation_tables
    keep = "natural_log_exp_and_others"
    orig = get_activation_tables("gen3")
    pinned = {k: (v if k == keep else set()) for k, v in orig.items()}
    bacc.get_activation_tables = lambda arch: pinned


def _build():
    _pin_act_table()
    nc = bacc.Bacc(
        "TRN2", target_bir_lowering=False, debug=False, num_devices=NCORES
    )
    d = {}
    def din(name, shape, dt):
        d[name] = nc.dram_tensor(name, list(shape), dt, kind="ExternalInput").ap()
    din("tvec", [V, E], F32)
    din("cvec", [V, E], F32)
    din("acwt", [E, DA], BF16)      # Ac_w.T
    din("atwt", [E, DA], BF16)      # At_w.T
    din("bcwt", [E, E], BF16)       # Bc_w.T
    din("rwt", [E, E], BF16)        # R_w.T
    din("rwt32", [E, E], F32)       # R_w.T fp32 (c2 path)
    din("acb", [DA, 1], F32)
    din("atb", [DA, 1], F32)
    din("bcb32", [E, 1], F32)
    din("rb32", [E, 1], F32)
    din("cit1", [C1, BL], I32)
    din("cit2", [C2, BL], I32)
    din("mlog1", [C1, BL], F32)
    din("mlog2", [C2, BL], F32)
    din("titg", [NB * T, NG], I32)
    z_dram = nc.dram_tensor("z_out", [BL * T, E], F32, kind="ExternalOutput").ap()

    with tile.TileContext(nc) as tc:
        with (
            tc.tile_pool(name="const", bufs=1) as cp,
            tc.tile_pool(name="work", bufs=2) as wp,
            tc.tile_pool(name="work3", bufs=3 * PB) as wp3,
            tc.tile_pool(name="psA", bufs=2, space="PSUM") as psA,
            tc.tile_pool(name="psB", bufs=1, space="PSUM") as psB,
            tc.tile_pool(name="psC", bufs=2, space="PSUM") as psC,
            tc.tile_pool(name="work4", bufs=4) as wp4,
        ):
            # ---- constants into SBUF ----
            idb = cp.tile([128, 128], BF16, tag="idb")
            make_identity(nc, idb[:])
            idf = cp.tile([128, 128], F32, tag="idf")
            make_identity(nc, idf[:])
            onesb = cp.tile([128, 1], BF16, tag="onesb")
            nc.gpsimd.memset(onesb[:], 1.0)
            ones_row32 = cp.tile([1, 128], F32, tag="onesr")
            nc.gpsimd.memset(ones_row32[:], 1.0)

            acwt = cp.tile([E, DA], BF16, tag="acwt")
            nc.sync.dma_start(acwt[:], d["acwt"][:])
            atwt = cp.tile([E, DA], BF16, tag="atwt")
            nc.sync.dma_start(atwt[:], d["atwt"][:])
            bcwt = cp.tile([E, E], BF16, tag="bcwt")
            nc.sync.dma_start(bcwt[:], d["bcwt"][:])
            rwt = cp.tile([E, E], BF16, tag="rwt")
            nc.sync.dma_start(rwt[:], d["rwt"][:])
            rwt32 = cp.tile([E, E], F32, tag="rwt32")
            nc.sync.dma_start(rwt32[:], d["rwt32"][:])
            acb = cp.tile([DA, 1], F32, tag="acb")
            nc.sync.dma_start(acb[:], d["acb"][:])
            atb = cp.tile([DA, 1], F32, tag="atb")
            nc.sync.dma_start(atb[:], d["atb"][:])
            bcb32 = cp.tile([E, 1], F32, tag="bcb32")
            nc.sync.dma_start(bcb32[:], d["bcb32"][:])
            rb32 = cp.tile([E, 1], F32, tag="rb32")
            nc.sync.dma_start(rb32[:], d["rb32"][:])
            cit1 = cp.tile([C1, BL], I32, tag="cit1")
            nc.sync.dma_start(cit1[:], d["cit1"][:])
            cit2 = cp.tile([C2, BL], I32, tag="cit2")
            nc.sync.dma_start(cit2[:], d["cit2"][:])
            mlog1 = cp.tile([C1, BL], F32, tag="mlog1")
            nc.sync.dma_start(mlog1[:], d["mlog1"][:])
            mlog2 = cp.tile([C2, BL], F32, tag="mlog2")
            nc.sync.dma_start(mlog2[:], d["mlog2"][:])
            titg = cp.tile([NB * T, NG], I32, tag="titg")
            nc.sync.dma_start(titg[:], d["titg"][:])

            # ---- one-time: c2b = broadcast(R_w @ Bc_b + R_b) (fp32 path) ----
            ps_c2 = psB.tile([E, 1], F32, space="PSUM", tag="grp")
            nc.tensor.matmul(ps_c2[:], lhsT=rwt32[:], rhs=bcb32[:])
            c2col = cp.tile([E, 1], F32, tag="c2col")
            nc.scalar.activation(c2col[:], ps_c2[:], AF.Identity, bias=rb32[:])
            ps_c2r = psB.tile([1, E], F32, space="PSUM", tag="grp")
            nc.tensor.matmul(ps_c2r[:], lhsT=c2col[:], rhs=idf[:])
            c2row = cp.tile([1, E], F32, tag="c2row")
            nc.scalar.copy(c2row[:], ps_c2r[:])
            ps_c2b = psB.tile([E, E], F32, space="PSUM", tag="grp")
            nc.tensor.matmul(ps_c2b[:], lhsT=ones_row32[:], rhs=c2row[:])
            c2b = cp.tile([E, E], F32, tag="c2b")
            nc.scalar.copy(c2b[:], ps_c2b[:])

            # ---- main loop ----
            for g in range(NG):
                # --- t path (whole group: 16 b x 8 t = 128 rows) ---
                tv = wp.tile([128, E], BF16, tag="tv")
                nc.gpsimd.indirect_dma_start(
                    out=tv[:], out_offset=None, in_=d["tvec"][:],
                    in_offset=bass.IndirectOffsetOnAxis(ap=titg[:, g:g + 1], axis=0),
                )
                ps_vT = psB.tile([E, 128], F32, space="PSUM", tag="grp")
                nc.tensor.matmul(ps_vT[:], lhsT=tv[:], rhs=idb[:])
                vT = wp.tile([E, 128], BF16, tag="vT")
                nc.scalar.copy(vT[:], ps_vT[:])
                ps_tvT = psB.tile([DA, 128], F32, space="PSUM", tag="grp")
                nc.tensor.matmul(ps_tvT[:], lhsT=atwt[:], rhs=vT[:])
                tvT = wp.tile([DA, 128], BF16, tag="tvT")
                nc.scalar.activation(tvT[:], ps_tvT[:], AF.Identity, bias=atb[:])
                ps_tv = psB.tile([128, DA], F32, space="PSUM", tag="grp")
                nc.tensor.matmul(ps_tv[:], lhsT=tvT[:], rhs=idb[0:DA, 0:DA])
                tsq = wp.tile([128, DA], BF16, tag="tsq")
                tn2 = wp.tile([128, 1], F32, tag="tn2")
                nc.scalar.activation(
                    tsq[:], ps_tv[:], AF.Square, accum_out=tn2[:],
                )
                # 1/sqrt(x) = exp(-0.5*ln(x)): keeps every ACT func in the
                # natural_log_exp table (a Sqrt would force 1.3us table
                # reloads next to each Exp)
                ltn = wp.tile([128, 1], F32, tag="ltn")
                nc.scalar.activation(ltn[:], tn2[:], AF.Ln)
                invtn = wp.tile([128, 1], F32, tag="invtn")
                nc.scalar.activation(invtn[:], ltn[:], AF.Exp, scale=-0.5)
                thbt = wp.tile([128, DA], BF16, tag="thbt")
                nc.vector.tensor_scalar_mul(thbt[:], ps_tv[:], invtn[:])
                ps_thT = psB.tile([DA, 128], F32, space="PSUM", tag="grp")
                nc.tensor.matmul(ps_thT[:], lhsT=thbt[:], rhs=idb[:])
                thT = wp.tile([DA, 128], BF16, tag="thT")
                nc.scalar.copy(thT[:], ps_thT[:])

                agA = wp.tile([C1, 128], BF16, tag="agA")
                agB = wp.tile([C2, 128], BF16, tag="agB")
                sTG = wp.tile([E, 128], BF16, tag="sTG")

                for blk in range(NB // PB):
                    us = []
                    ps_nT4 = psA.tile([C1, PB * 18], F32, space="PSUM", tag="nT")
                    for j in range(PB):
                        i = blk * PB + j
                        b = g * NB + i
                        u1 = wp3.tile([C1, E], BF16, tag="u1")
                        nc.gpsimd.indirect_dma_start(
                            out=u1[:], out_offset=None, in_=d["cvec"][:],
                            in_offset=bass.IndirectOffsetOnAxis(
                                ap=cit1[:, b:b + 1], axis=0),
                        )
                        u2 = wp3.tile([C2, E], BF16, tag="u2")
                        nc.gpsimd.indirect_dma_start(
                            out=u2[:], out_offset=None, in_=d["cvec"][:],
                            in_offset=bass.IndirectOffsetOnAxis(
                                ap=cit2[:, b:b + 1], axis=0),
                        )
                        us.append((u1, u2))
                        # u_T = [u1; u2]^T  -> [E, C]
                        ps_uT = psA.tile([E, C], F32, space="PSUM", tag="uT")
                        nc.tensor.matmul(ps_uT[:, 0:C1], lhsT=u1[:], rhs=idb[:])
                        nc.tensor.matmul(ps_uT[:, C1:C], lhsT=u2[:],
                                         rhs=idb[0:C2, 0:C2])
                        uT = wp4.tile([E, C], BF16, tag="uT_sb")
                        nc.scalar.copy(uT[:, 0:100], ps_uT[:, 0:100])
                        nc.vector.tensor_copy(uT[:, 100:C], ps_uT[:, 100:C])
                        # c_vec DA-major [60, C] (+bias via DVE on copy-out)
                        ps_cvT = psB.tile([DA, C], F32, space="PSUM", tag="cvT")
                        nc.tensor.matmul(ps_cvT[:], lhsT=acwt[:], rhs=uT[:])
                        cvT = wp4.tile([DA, C], BF16, tag="cvT_sb")
                        nc.vector.tensor_scalar(
                            out=cvT[:], in0=ps_cvT[:], scalar1=acb[:],
                            scalar2=None, op0=OP.add,
                        )
                        sq = wp4.tile([DA, C], BF16, tag="sq")
                        nc.vector.scalar_tensor_tensor(
                            out=sq[:], in0=cvT[:], scalar=1.0, in1=cvT[:],
                            op0=OP.mult, op1=OP.mult,
                        )
                        # per-b columns of ps_nT4: [18j,18j+8) num1,
                        # [18j+8,18j+16) num2 (rows<72), 18j+16 cn1, 18j+17 cn2
                        o = 18 * j
                        nc.tensor.matmul(ps_nT4[:, o + 16:o + 17],
                                         lhsT=sq[:, 0:C1], rhs=onesb[0:DA, :])
                        nc.tensor.matmul(ps_nT4[0:C2, o + 17:o + 18],
                                         lhsT=sq[:, C1:C], rhs=onesb[0:DA, :])
                        nc.tensor.matmul(ps_nT4[:, o:o + T], lhsT=cvT[:, 0:C1],
                                         rhs=thT[:, i * T:(i + 1) * T])
                        nc.tensor.matmul(ps_nT4[0:C2, o + T:o + 2 * T],
                                         lhsT=cvT[:, C1:C],
                                         rhs=thT[:, i * T:(i + 1) * T])
                    # batched invcn = exp(-0.5*ln(cn^2)) for all PB rows
                    lcn = wp4.tile([C1, PB, 2], F32, tag="lcn")
                    cn_view = ps_nT4[:].rearrange("p (b k) -> p b k", k=18)[:, :, 16:18]
                    nc.scalar.activation(lcn[:], cn_view, AF.Ln)
                    invcn = wp4.tile([C1, PB * 2], F32, tag="invcn")
                    nc.scalar.activation(
                        invcn[:], lcn[:].rearrange("p b k -> p (b k)"),
                        AF.Exp, scale=-0.5)
                    for j in range(PB):
                        i = blk * PB + j
                        b = g * NB + i
                        u1, u2 = us[j]
                        o = 18 * j
                        # attn_unnorm = exp(num*invcn + masklog)
                        nc.scalar.activation(
                            agA[:, i * T:(i + 1) * T], ps_nT4[:, o:o + T],
                            AF.Exp,
                            bias=mlog1[:, b:b + 1],
                            scale=invcn[:, 2 * j:2 * j + 1],
                        )
                        nc.scalar.activation(
                            agB[:, i * T:(i + 1) * T],
                            ps_nT4[0:C2, o + T:o + 2 * T], AF.Exp,
                            bias=mlog2[:, b:b + 1],
                            scale=invcn[0:C2, 2 * j + 1:2 * j + 2],
                        )
                        # s_T = u^T @ attn  [E, 8]
                        ps_sT = psC.tile([E, T], F32, space="PSUM", tag="sT")
                        nc.tensor.matmul(ps_sT[:], lhsT=u1[:],
                                         rhs=agA[:, i * T:(i + 1) * T],
                                         start=True, stop=False)
                        nc.tensor.matmul(ps_sT[:], lhsT=u2[:],
                                         rhs=agB[:, i * T:(i + 1) * T],
                                         start=False, stop=True)
                        nc.vector.tensor_copy(sTG[:, i * T:(i + 1) * T],
                                              ps_sT[:])

                # --- group tail: Sigma, z path ---
                ps_sum = psB.tile([128, 1], F32, space="PSUM", tag="grp")
                nc.tensor.matmul(ps_sum[:], lhsT=agA[:], rhs=onesb[0:C1, :],
                                 start=True, stop=False)
                nc.tensor.matmul(ps_sum[:], lhsT=agB[:], rhs=onesb[0:C2, :],
                                 start=False, stop=True)
                invS = wp.tile([128, 1], F32, tag="invS")
                nc.vector.reciprocal(invS[:], ps_sum[:])

                ps_yT = psB.tile([E, 128], F32, space="PSUM", tag="grp")
                nc.tensor.matmul(ps_yT[:], lhsT=bcwt[:], rhs=sTG[:])
                yT = wp.tile([E, 128], BF16, tag="yT")
                nc.scalar.copy(yT[:], ps_yT[:])
                ps_zT = psB.tile([E, 128], F32, space="PSUM", tag="grp")
                nc.tensor.matmul(ps_zT[:], lhsT=rwt[:], rhs=yT[:])
                zT = wp.tile([E, 128], BF16, tag="zT")
                nc.scalar.copy(zT[:], ps_zT[:])
                ps_z = psB.tile([128, E], F32, space="PSUM", tag="grp")
                nc.tensor.matmul(ps_z[:], lhsT=zT[:], rhs=idb[:])
                zout = wp.tile([128, E], F32, tag="zout")
                nc.vector.scalar_tensor_tensor(
                    out=zout[:], in0=ps_z[:], scalar=invS[:], in1=c2b[:],
                    op0=OP.mult, op1=OP.add,
                )
                nc.sync.dma_start(z_dram[g * 128:(g + 1) * 128, :], zout[:])

    nc.compile()
    return nc


def _prep_core_inputs(inputs, k):
    bf = ml_dtypes.bfloat16
    sl = slice(k * BL, (k + 1) * BL)
    tit = np.ascontiguousarray(
        inputs["batch_titems"][sl].astype(np.int32).reshape(NG, NB * T).T)
    cit = inputs["batch_citems"][sl].astype(np.int32).T
    mlog = np.where(inputs["mask_pad_ids"][sl], NEG, 0.0).astype(np.float32).T
    m = {
        "tvec": np.asarray(inputs["tvec_w"], dtype=np.float32),
        "cvec": np.asarray(inputs["cvec_w"], dtype=np.float32),
        "acwt": np.ascontiguousarray(inputs["Ac_w"].T).astype(bf),
        "atwt": np.ascontiguousarray(inputs["At_w"].T).astype(bf),
        "bcwt": np.ascontiguousarray(inputs["Bc_w"].T).astype(bf),
        "rwt": np.ascontiguousarray(inputs["R_w"].T).astype(bf),
        "rwt32": np.ascontiguousarray(inputs["R_w"].T).astype(np.float32),
        "acb": np.asarray(inputs["Ac_b"], dtype=np.float32).reshape(DA, 1),
        "atb": np.asarray(inputs["At_b"], dtype=np.float32).reshape(DA, 1),
        "bcb32": np.asarray(inputs["Bc_b"], dtype=np.float32).reshape(E, 1),
        "rb32": np.asarray(inputs["R_b"], dtype=np.float32).reshape(E, 1),
        "cit1": np.ascontiguousarray(cit[0:C1]),
        "cit2": np.ascontiguousarray(cit[C1:C]),
        "mlog1": np.ascontiguousarray(mlog[0:C1]),
        "mlog2": np.ascontiguousarray(mlog[C1:C]),
        "titg": tit,
    }
    return m


def _install_profile_hook():
    """Dev-only: register the axon NTFF hook missing from this image."""
    import sys
    import types
    try:
        import antenv.axon_hooks  # noqa: F401
        return
    except ImportError:
        pass
    from trn_agent_boot.trn_boot import _ntff_profile_via_ctypes
    hook = _ntff_profile_via_ctypes("/opt/axon/libaxon_pjrt.so")
    mod = types.ModuleType("antenv.axon_hooks")
    mod._hook = hook
    mod.set_axon_ntff_profile_hook = lambda h: setattr(mod, "_hook", h)
    mod.get_axon_ntff_profile_hook = lambda: mod._hook
    sys.modules["antenv.axon_hooks"] = mod
    import antenv
    antenv.axon_hooks = mod


def kernel(**inputs) -> np.ndarray:
    if "nc" not in _CACHE:
        _CACHE["nc"] = _build()
    nc = _CACHE["nc"]
    inputs = {k: np.asarray(v) for k, v in inputs.items()}
    in_maps = [_prep_core_inputs(inputs, k) for k in range(NCORES)]
    trace = bool(int(os.environ.get("KERNEL_TRACE", "0")))
    kw = {}
    if trace:
        try:
            _install_profile_hook()
            import concourse.bass_utils as _bu
            _bu.upload_artifacts = lambda d: d
            tdir = os.environ.get("KERNEL_TRACE_DIR", "/root/problem/_trace")
            os.makedirs(tdir, exist_ok=True)
            kw["tmpdir"] = tdir
        except Exception as e:  # profiling is best-effort
            print(f"trace setup failed: {e}")
            trace = False
    res = run_bass_kernel_spmd(
        nc, in_maps, list(range(NCORES)), trace=trace, **kw,
    )
    _CACHE["last_result"] = res
    z = np.concatenate(
        [res.results[k]["z_out"].reshape(BL, T, E) for k in range(NCORES)], axis=0
    )
    return z.astype(np.float32)


# revision 13
# speedup vs baseline: 2.1879x; 1.0188x over previous
"""AttentiveItemToVec Trainium2 kernel (8 NeuronCores, batch-parallel).

Math (per batch row b):
  v = tvec_w[titems[b]]            [T,E]     (gather)
  u = cvec_w[citems[b]]            [C,E]     (gather)
  t_vec = v @ At_w.T + At_b        [T,DA]
  c_vec = u @ Ac_w.T + Ac_b        [C,DA]
  cos   = (t_vec/|t_vec|) . (c_vec/|c_vec|)   [T,C]
  attn  = softmax(mask(cos))       [T,C]
  z     = (attn @ (u @ Bc_w.T + Bc_b)) @ R_w.T + R_b
        = (attn@u) @ (R_w@Bc_w).T ... expanded here as:
          s = attn_unnorm @ u;  z = ((s/Sigma) @ Bc_w.T) @ R_w.T + (R_w@Bc_b + R_b)
  (softmax row-sums fold Bc_b through exactly since attn rows sum to 1)

Layout strategy per core (512 batch rows, groups of 16):
  - u gathered row-major [C,128] (c on partitions), PE-transposed to u_T [128,C]
  - c_vec computed DA-major [60,C]; cn^2 via ones-matmul (C-major out)
  - cos/softmax entirely C-major; exp does (num*invcn + masklog) in one ACT op
  - s_T accumulated E-major; group-level z matmuls; final transpose + 1/Sigma
"""

import os
import numpy as np
import ml_dtypes

import concourse.bass as bass
import concourse.bacc as bacc
import concourse.mybir as mybir
import concourse.tile as tile
from concourse.bass_utils import run_bass_kernel_spmd
from concourse.masks import make_identity

F32 = mybir.dt.float32
BF16 = mybir.dt.bfloat16
I32 = mybir.dt.int32
AF = mybir.ActivationFunctionType
OP = mybir.AluOpType

V, E, DA = 100000, 128, 60
B, T, C = 4096, 8, 200
NCORES = 8
BL = B // NCORES          # 512 local batch rows
NB = 16                   # batch rows per group (NB*T = 128 partitions)
NG = BL // NB             # 32 groups
PB = 4                    # batch rows gathered per indirect DMA
C1, C2 = 128, C - 128     # C chunking: 128 + 72
NEG = -1e30

_CACHE: dict = {}


def _pin_act_table():
    """Force every activation onto the natural_log_exp_and_others table.

    All ACT funcs used here (Copy/Identity/Square/Ln/Exp) live in that one
    table, but the table chooser picks the first table containing each
    function, which makes Exp->Ln sequences thrash 1.28us ACT_TABLE_LOADs.
    Emptying the other sets (names/positions preserved so act_func_set ids
    stay valid) pins the choice; one load total.
    """
    from concourse.hw_specs import get_activation_tables
    keep = "natural_log_exp_and_others"
    orig = get_activation_tables("gen3")
    pinned = {k: (v if k == keep else set()) for k, v in orig.items()}
    bacc.get_activation_tables = lambda arch: pinned


def _build():
    _pin_act_table()
    nc = bacc.Bacc(
        "TRN2", target_bir_lowering=False, debug=False, num_devices=NCORES
    )
    d = {}
    def din(name, shape, dt):
        d[name] = nc.dram_tensor(name, list(shape), dt, kind="ExternalInput").ap()
    din("tvec", [V, E], F32)
    din("cvec", [V, E], F32)
    din("acwt", [E, DA], BF16)      # Ac_w.T
    din("atwt", [E, DA], BF16)      # At_w.T
    din("bcwt", [E, E], BF16)       # Bc_w.T
    din("rwt", [E, E], BF16)        # R_w.T
    din("rwt32", [E, E], F32)       # R_w.T fp32 (c2 path)
    din("acb", [DA, 1], F32)
    din("atb", [DA, 1], F32)
    din("bcb32", [E, 1], F32)
    din("rb32", [E, 1], F32)
    din("cit1", [C1, BL], I32)
    din("cit2", [C2, BL], I32)
    din("mlog1", [C1, BL], F32)
    din("mlog2", [C2, BL], F32)
    din("titg", [NB * T, NG], I32)
    z_dram = nc.dram_tensor("z_out", [BL * T, E], F32, kind="ExternalOutput").ap()

    with tile.TileContext(nc) as tc:
        with (
            tc.tile_pool(name="const", bufs=1) as cp,
            tc.tile_pool(name="work", bufs=2) as wp,
            tc.tile_pool(name="work3", bufs=5 * PB) as wp3,
            tc.tile_pool(name="psA", bufs=2, space="PSUM") as psA,
            tc.tile_pool(name="psB", bufs=1, space="PSUM") as psB,
            tc.tile_pool(name="psC", bufs=1, space="PSUM") as psC,
            tc.tile_pool(name="work4", bufs=6) as wp4,
            tc.tile_pool(name="psD", bufs=3, space="PSUM") as psD,
        ):
            # ---- constants into SBUF ----
            idb = cp.tile([128, 128], BF16, tag="idb")
            make_identity(nc, idb[:])
            idf = cp.tile([128, 128], F32, tag="idf")
            make_identity(nc, idf[:])
            onesb = cp.tile([128, 1], BF16, tag="onesb")
            nc.gpsimd.memset(onesb[:], 1.0)
            ones_row32 = cp.tile([1, 128], F32, tag="onesr")
            nc.gpsimd.memset(ones_row32[:], 1.0)

            acwt = cp.tile([E, DA], BF16, tag="acwt")
            nc.sync.dma_start(acwt[:], d["acwt"][:])
            atwt = cp.tile([E, DA], BF16, tag="atwt")
            nc.sync.dma_start(atwt[:], d["atwt"][:])
            bcwt = cp.tile([E, E], BF16, tag="bcwt")
            nc.sync.dma_start(bcwt[:], d["bcwt"][:])
            rwt = cp.tile([E, E], BF16, tag="rwt")
            nc.sync.dma_start(rwt[:], d["rwt"][:])
            rwt32 = cp.tile([E, E], F32, tag="rwt32")
            nc.sync.dma_start(rwt32[:], d["rwt32"][:])
            acb = cp.tile([DA, 1], F32, tag="acb")
            nc.sync.dma_start(acb[:], d["acb"][:])
            atb = cp.tile([DA, 1], F32, tag="atb")
            nc.sync.dma_start(atb[:], d["atb"][:])
            bcb32 = cp.tile([E, 1], F32, tag="bcb32")
            nc.sync.dma_start(bcb32[:], d["bcb32"][:])
            rb32 = cp.tile([E, 1], F32, tag="rb32")
            nc.sync.dma_start(rb32[:], d["rb32"][:])
            cit1 = cp.tile([C1, BL], I32, tag="cit1")
            nc.sync.dma_start(cit1[:], d["cit1"][:])
            cit2 = cp.tile([C2, BL], I32, tag="cit2")
            nc.sync.dma_start(cit2[:], d["cit2"][:])
            mlog1 = cp.tile([C1, BL], F32, tag="mlog1")
            nc.sync.dma_start(mlog1[:], d["mlog1"][:])
            mlog2 = cp.tile([C2, BL], F32, tag="mlog2")
            nc.sync.dma_start(mlog2[:], d["mlog2"][:])
            titg = cp.tile([NB * T, NG], I32, tag="titg")
            nc.sync.dma_start(titg[:], d["titg"][:])

            # ---- one-time: c2b = broadcast(R_w @ Bc_b + R_b) (fp32 path) ----
            ps_c2 = psB.tile([E, 1], F32, space="PSUM", tag="grp")
            nc.tensor.matmul(ps_c2[:], lhsT=rwt32[:], rhs=bcb32[:])
            c2col = cp.tile([E, 1], F32, tag="c2col")
            nc.scalar.activation(c2col[:], ps_c2[:], AF.Identity, bias=rb32[:])
            ps_c2r = psB.tile([1, E], F32, space="PSUM", tag="grp")
            nc.tensor.matmul(ps_c2r[:], lhsT=c2col[:], rhs=idf[:])
            c2row = cp.tile([1, E], F32, tag="c2row")
            nc.scalar.copy(c2row[:], ps_c2r[:])
            ps_c2b = psB.tile([E, E], F32, space="PSUM", tag="grp")
            nc.tensor.matmul(ps_c2b[:], lhsT=ones_row32[:], rhs=c2row[:])
            c2b = cp.tile([E, E], F32, tag="c2b")
            nc.scalar.copy(c2b[:], ps_c2b[:])

            # ---- main loop ----
            for g in range(NG):
                # --- t path (whole group: 16 b x 8 t = 128 rows) ---
                tv = wp.tile([128, E], BF16, tag="tv")
                nc.gpsimd.indirect_dma_start(
                    out=tv[:], out_offset=None, in_=d["tvec"][:],
                    in_offset=bass.IndirectOffsetOnAxis(ap=titg[:, g:g + 1], axis=0),
                )
                ps_vT = psB.tile([E, 128], F32, space="PSUM", tag="grp")
                nc.tensor.matmul(ps_vT[:], lhsT=tv[:], rhs=idb[:])
                vT = wp.tile([E, 128], BF16, tag="vT")
                nc.scalar.copy(vT[:], ps_vT[:])
                ps_tvT = psB.tile([DA, 128], F32, space="PSUM", tag="grp")
                nc.tensor.matmul(ps_tvT[:], lhsT=atwt[:], rhs=vT[:])
                tvT = wp.tile([DA, 128], BF16, tag="tvT")
                nc.scalar.activation(tvT[:], ps_tvT[:], AF.Identity, bias=atb[:])
                ps_tv = psB.tile([128, DA], F32, space="PSUM", tag="grp")
                nc.tensor.matmul(ps_tv[:], lhsT=tvT[:], rhs=idb[0:DA, 0:DA])
                tsq = wp.tile([128, DA], BF16, tag="tsq")
                tn2 = wp.tile([128, 1], F32, tag="tn2")
                nc.scalar.activation(
                    tsq[:], ps_tv[:], AF.Square, accum_out=tn2[:],
                )
                # 1/sqrt(x) = exp(-0.5*ln(x)): keeps every ACT func in the
                # natural_log_exp table (a Sqrt would force 1.3us table
                # reloads next to each Exp)
                ltn = wp.tile([128, 1], F32, tag="ltn")
                nc.scalar.activation(ltn[:], tn2[:], AF.Ln)
                invtn = wp.tile([128, 1], F32, tag="invtn")
                nc.scalar.activation(invtn[:], ltn[:], AF.Exp, scale=-0.5)
                thbt = wp.tile([128, DA], BF16, tag="thbt")
                nc.vector.tensor_scalar_mul(thbt[:], ps_tv[:], invtn[:])
                ps_thT = psB.tile([DA, 128], F32, space="PSUM", tag="grp")
                nc.tensor.matmul(ps_thT[:], lhsT=thbt[:], rhs=idb[:])
                thT = wp.tile([DA, 128], BF16, tag="thT")
                nc.scalar.copy(thT[:], ps_thT[:])

                agA = wp.tile([C1, 128], BF16, tag="agA")
                agB = wp.tile([C2, 128], BF16, tag="agB")
                sTG = wp.tile([E, 128], BF16, tag="sTG")

                for blk in range(NB // PB):
                    us = []
                    ps_nT4 = psD.tile([C1, PB * 18], F32, space="PSUM", tag="nT")
                    for j in range(PB):
                        i = blk * PB + j
                        b = g * NB + i
                        u1 = wp3.tile([C1, E], BF16, tag="u1")
                        nc.gpsimd.indirect_dma_start(
                            out=u1[:], out_offset=None, in_=d["cvec"][:],
                            in_offset=bass.IndirectOffsetOnAxis(
                                ap=cit1[:, b:b + 1], axis=0),
                        )
                        u2 = wp3.tile([C2, E], BF16, tag="u2")
                        nc.gpsimd.indirect_dma_start(
                            out=u2[:], out_offset=None, in_=d["cvec"][:],
                            in_offset=bass.IndirectOffsetOnAxis(
                                ap=cit2[:, b:b + 1], axis=0),
                        )
                        us.append((u1, u2))
                        # u_T = [u1; u2]^T  -> [E, C]
                        ps_uT = psA.tile([E, C], F32, space="PSUM", tag="uT")
                        nc.tensor.matmul(ps_uT[:, 0:C1], lhsT=u1[:], rhs=idb[:])
                        nc.tensor.matmul(ps_uT[:, C1:C], lhsT=u2[:],
                                         rhs=idb[0:C2, 0:C2])
                        uT = wp4.tile([E, C], BF16, tag="uT_sb")
                        nc.scalar.copy(uT[:, 0:100], ps_uT[:, 0:100])
                        nc.vector.tensor_copy(uT[:, 100:C], ps_uT[:, 100:C])
                        # c_vec DA-major [60, C] (+bias via DVE on copy-out)
                        ps_cvT = psB.tile([DA, C], F32, space="PSUM", tag="cvT")
                        nc.tensor.matmul(ps_cvT[:], lhsT=acwt[:], rhs=uT[:])
                        cvT = wp4.tile([DA, C], BF16, tag="cvT_sb")
                        nc.vector.tensor_scalar(
                            out=cvT[:], in0=ps_cvT[:], scalar1=acb[:],
                            scalar2=None, op0=OP.add,
                        )
                        sq = wp4.tile([DA, C], BF16, tag="sq")
                        nc.vector.scalar_tensor_tensor(
                            out=sq[:], in0=cvT[:], scalar=1.0, in1=cvT[:],
                            op0=OP.mult, op1=OP.mult,
                        )
                        # per-b columns of ps_nT4: [18j,18j+8) num1,
                        # [18j+8,18j+16) num2 (rows<72), 18j+16 cn1, 18j+17 cn2
                        o = 18 * j
                        nc.tensor.matmul(ps_nT4[:, o + 16:o + 17],
                                         lhsT=sq[:, 0:C1], rhs=onesb[0:DA, :])
                        nc.tensor.matmul(ps_nT4[0:C2, o + 17:o + 18],
                                         lhsT=sq[:, C1:C], rhs=onesb[0:DA, :])
                        nc.tensor.matmul(ps_nT4[:, o:o + T], lhsT=cvT[:, 0:C1],
                                         rhs=thT[:, i * T:(i + 1) * T])
                        nc.tensor.matmul(ps_nT4[0:C2, o + T:o + 2 * T],
                                         lhsT=cvT[:, C1:C],
                                         rhs=thT[:, i * T:(i + 1) * T])
                    # batched invcn = exp(-0.5*ln(cn^2)) for all PB rows
                    lcn = wp4.tile([C1, PB, 2], F32, tag="lcn")
                    cn_view = ps_nT4[:].rearrange("p (b k) -> p b k", k=18)[:, :, 16:18]
                    nc.scalar.activation(lcn[:], cn_view, AF.Ln)
                    invcn = wp4.tile([C1, PB * 2], F32, tag="invcn")
                    nc.scalar.activation(
                        invcn[:], lcn[:].rearrange("p b k -> p (b k)"),
                        AF.Exp, scale=-0.5)
                    for j in range(PB):
                        i = blk * PB + j
                        b = g * NB + i
                        u1, u2 = us[j]
                        o = 18 * j
                        # attn_unnorm = exp(num*invcn + masklog)
                        nc.scalar.activation(
                            agA[:, i * T:(i + 1) * T], ps_nT4[:, o:o + T],
                            AF.Exp,
                            bias=mlog1[:, b:b + 1],
                            scale=invcn[:, 2 * j:2 * j + 1],
                        )
                        nc.scalar.activation(
                            agB[:, i * T:(i + 1) * T],
                            ps_nT4[0:C2, o + T:o + 2 * T], AF.Exp,
                            bias=mlog2[:, b:b + 1],
                            scale=invcn[0:C2, 2 * j + 1:2 * j + 2],
                        )
                        # s_T = u^T @ attn  [E, 8]
                        ps_sT = psC.tile([E, T], F32, space="PSUM", tag="sT")
                        nc.tensor.matmul(ps_sT[:], lhsT=u1[:],
                                         rhs=agA[:, i * T:(i + 1) * T],
                                         start=True, stop=False)
                        nc.tensor.matmul(ps_sT[:], lhsT=u2[:],
                                         rhs=agB[:, i * T:(i + 1) * T],
                                         start=False, stop=True)
                        nc.vector.tensor_copy(sTG[:, i * T:(i + 1) * T],
                                              ps_sT[:])

                # --- group tail: Sigma, z path ---
                ps_sum = psB.tile([128, 1], F32, space="PSUM", tag="grp")
                nc.tensor.matmul(ps_sum[:], lhsT=agA[:], rhs=onesb[0:C1, :],
                                 start=True, stop=False)
                nc.tensor.matmul(ps_sum[:], lhsT=agB[:], rhs=onesb[0:C2, :],
                                 start=False, stop=True)
                invS = wp.tile([128, 1], F32, tag="invS")
                nc.vector.reciprocal(invS[:], ps_sum[:])

                ps_yT = psB.tile([E, 128], F32, space="PSUM", tag="grp")
                nc.tensor.matmul(ps_yT[:], lhsT=bcwt[:], rhs=sTG[:])
                yT = wp.tile([E, 128], BF16, tag="yT")
                nc.scalar.copy(yT[:], ps_yT[:])
                ps_zT = psB.tile([E, 128], F32, space="PSUM", tag="grp")
                nc.tensor.matmul(ps_zT[:], lhsT=rwt[:], rhs=yT[:])
                zT = wp.tile([E, 128], BF16, tag="zT")
                nc.scalar.copy(zT[:], ps_zT[:])
                ps_z = psB.tile([128, E], F32, space="PSUM", tag="grp")
                nc.tensor.matmul(ps_z[:], lhsT=zT[:], rhs=idb[:])
                zout = wp.tile([128, E], F32, tag="zout")
                nc.vector.scalar_tensor_tensor(
                    out=zout[:], in0=ps_z[:], scalar=invS[:], in1=c2b[:],
                    op0=OP.mult, op1=OP.add,
                )
                nc.sync.dma_start(z_dram[g * 128:(g + 1) * 128, :], zout[:])

    nc.compile()
    return nc


def _prep_core_inputs(inputs, k):
    bf = ml_dtypes.bfloat16
    sl = slice(k * BL, (k + 1) * BL)
    tit = np.ascontiguousarray(
        inputs["batch_titems"][sl].astype(np.int32).reshape(NG, NB * T).T)
    cit = inputs["batch_citems"][sl].astype(np.int32).T
    mlog = np.where(inputs["mask_pad_ids"][sl], NEG, 0.0).astype(np.float32).T
    m = {
        "tvec": np.asarray(inputs["tvec_w"], dtype=np.float32),
        "cvec": np.asarray(inputs["cvec_w"], dtype=np.float32),
        "acwt": np.ascontiguousarray(inputs["Ac_w"].T).astype(bf),
        "atwt": np.ascontiguousarray(inputs["At_w"].T).astype(bf),
        "bcwt": np.ascontiguousarray(inputs["Bc_w"].T).astype(bf),
        "rwt": np.ascontiguousarray(inputs["R_w"].T).astype(bf),
        "rwt32": np.ascontiguousarray(inputs["R_w"].T).astype(np.float32),
        "acb": np.asarray(inputs["Ac_b"], dtype=np.float32).reshape(DA, 1),
        "atb": np.asarray(inputs["At_b"], dtype=np.float32).reshape(DA, 1),
        "bcb32": np.asarray(inputs["Bc_b"], dtype=np.float32).reshape(E, 1),
        "rb32": np.asarray(inputs["R_b"], dtype=np.float32).reshape(E, 1),
        "cit1": np.ascontiguousarray(cit[0:C1]),
        "cit2": np.ascontiguousarray(cit[C1:C]),
        "mlog1": np.ascontiguousarray(mlog[0:C1]),
        "mlog2": np.ascontiguousarray(mlog[C1:C]),
        "titg": tit,
    }
    return m


def _install_profile_hook():
    """Dev-only: register the axon NTFF hook missing from this image."""
    import sys
    import types
    try:
        import antenv.axon_hooks  # noqa: F401
        return
    except ImportError:
        pass
    from trn_agent_boot.trn_boot import _ntff_profile_via_ctypes
    hook = _ntff_profile_via_ctypes("/opt/axon/libaxon_pjrt.so")
    mod = types.ModuleType("antenv.axon_hooks")
    mod._hook = hook
    mod.set_axon_ntff_profile_hook = lambda h: setattr(mod, "_hook", h)
    mod.get_axon_ntff_profile_hook = lambda: mod._hook
    sys.modules["antenv.axon_hooks"] = mod
    import antenv
    antenv.axon_hooks = mod


def kernel(**inputs) -> np.ndarray:
    if "nc" not in _CACHE:
        _CACHE["nc"] = _build()
    nc = _CACHE["nc"]
    inputs = {k: np.asarray(v) for k, v in inputs.items()}
    in_maps = [_prep_core_inputs(inputs, k) for k in range(NCORES)]
    trace = bool(int(os.environ.get("KERNEL_TRACE", "0")))
    kw = {}
    if trace:
        try:
            _install_profile_hook()
            import concourse.bass_utils as _bu
            _bu.upload_artifacts = lambda d: d
            tdir = os.environ.get("KERNEL_TRACE_DIR", "/root/problem/_trace")
            os.makedirs(tdir, exist_ok=True)
            kw["tmpdir"] = tdir
        except Exception as e:  # profiling is best-effort
            print(f"trace setup failed: {e}")
            trace = False
    res = run_bass_kernel_spmd(
        nc, in_maps, list(range(NCORES)), trace=trace, **kw,
    )
    _CACHE["last_result"] = res
    z = np.concatenate(
        [res.results[k]["z_out"].reshape(BL, T, E) for k in range(NCORES)], axis=0
    )
    return z.astype(np.float32)
